# revision 8
# baseline (speedup 1.0000x reference)
"""DistanceAttention Trainium2 kernel.

Computes, for x:[B,T,D]:
    v    = x @ W_in.T + b_in
    attn = exp((-|i-j| + padding_mask) / e)        # [B,T,T], no softmax
    out  = attn @ v

Key fact: attn factors as exp(-|i-j|/e) * exp(mask_j/e).  The distance
kernel r^|i-j| (r = exp(-1/e) ~= 0.692) underflows fp32 (< 1e-21) for
|i-j| >= 128, so attn is numerically block-tridiagonal with three
CONSTANT 128x128 blocks shared by every row-block, every batch, every
core.  The t x t matmul collapses to 3 small matmuls per 128-row block.

Sharding: batch(4) x seq-half(2) -> 8 cores, each owning 2048 rows plus
a 128-row halo on each side.  exp(mask/e) folds into a per-row scale on
v; phantom halo rows get scale 0.  No cross-core communication.
"""

import numpy as np

B, T, D = 4, 4096, 256
NCORES = 8
THALF = T // 2  # rows owned per core
HALO = 128
LOC = THALF + 2 * HALO  # local rows incl. halo
NBLK = LOC // 128  # 18 local 128-row blocks
E = float(np.e)

_CACHE: dict = {}

# bias injection: "k1" = K=1 matmul(ones[1,128], bias[1,256]);
# "row0" = K=128 matmul with a row-0-selector weight; "none" = skip (b_in==0)
BIAS_MODE = "row0"
MD_INLINE = True  # decay blocks as NEFF Const vs ExternalInput


def _decay_blocks() -> np.ndarray:
    """lhsT-layout decay blocks [128, 3*128]: L | 0 | R.

    matmul(out, lhsT, rhs) computes out[p,n] = sum_q lhsT[q,p] rhs[q,n].
    Out-block m needs  A_L @ v[m-1] + A_0 @ v[m] + A_R @ v[m+1]  with
      A_L[p,q] = r^(128+p-q),  A_0[p,q] = r^|p-q|,  A_R[p,q] = r^(128+q-p)
    so lhsT_L[q,p] = A_L[p,q] etc.  Entries are computed exactly like the
    reference: exp(-dist/e) in fp32.
    """
    i = np.arange(128, dtype=np.float64)
    dL = 128.0 + i[None, :] - i[:, None]  # lhsT_L[a,b] = r^(128+b-a)
    d0 = np.abs(i[:, None] - i[None, :])
    dR = 128.0 + i[:, None] - i[None, :]  # lhsT_R[a,b] = r^(128+a-b)
    dist = np.concatenate([dL, d0, dR], axis=1)
    tg = (-dist.astype(np.float32)) / np.float32(E)
    return np.exp(tg).astype(np.float32)


def _build():
    import concourse.bacc as bacc
    import concourse.mybir as mybir
    from concourse.bass import ts
    from concourse.tile import TileContext

    fp = mybir.dt.float32
    nc = bacc.Bacc(None, target_bir_lowering=False, debug=False)

    xT = nc.dram_tensor("xT", [2, 128, LOC], fp, kind="ExternalInput")
    wT = nc.dram_tensor("wT", [2, 128, D], fp, kind="ExternalInput")
    bias = nc.dram_tensor("bias", [1, D], fp, kind="ExternalInput")
    s = nc.dram_tensor("s", [128, NBLK], fp, kind="ExternalInput")
    out = nc.dram_tensor("out", [THALF, D], fp, kind="ExternalOutput")
    if MD_INLINE:
        md = nc.inline_tensor(_decay_blocks(), name="mdecay")
    else:
        md = nc.dram_tensor("mdecay", [128, 3 * 128], fp, kind="ExternalInput")

    with TileContext(nc) as tc:
        with (
            tc.tile_pool(name="const", bufs=1) as cpool,
            tc.tile_pool(name="vpool", bufs=NBLK) as vpool,
            tc.tile_pool(name="opool", bufs=3) as opool,
            tc.tile_pool(name="ppsum", bufs=3, space="PSUM") as ppsum,
            tc.tile_pool(name="dpsum", bufs=4, space="PSUM") as dpsum,
        ):
            xT_sb = []
            for k in range(2):
                t = cpool.tile([128, LOC], fp, tag=f"xT{k}")
                nc.sync.dma_start(out=t[:], in_=xT[k])
                xT_sb.append(t)
            wT_sb = cpool.tile([128, 2 * D], fp, tag="wT")
            for k in range(2):
                nc.sync.dma_start(out=wT_sb[:, ts(k, D)], in_=wT[k])
            s_sb = cpool.tile([128, NBLK], fp, tag="s")
            nc.sync.dma_start(out=s_sb[:], in_=s[:])
            md_sb = cpool.tile([128, 3 * 128], fp, tag="md")
            nc.sync.dma_start(out=md_sb[:], in_=md[:])
            if BIAS_MODE == "k1":
                bias_sb = cpool.tile([1, D], fp, tag="bias")
                nc.sync.dma_start(out=bias_sb[:], in_=bias[:])
                ones_sb = cpool.tile([1, 128], fp, tag="ones")
                nc.vector.memset(ones_sb[:], 1.0)
            elif BIAS_MODE == "row0":
                # bias in row 0 of a [128,D] tile; selector weight has
                # row 0 all-ones.  out += sel.T @ bias128 adds bias[n] to
                # every partition without a K=1 fp32 matmul.
                bias_sb = cpool.tile([128, D], fp, tag="bias")
                nc.vector.memset(bias_sb[:], 0.0)
                nc.sync.dma_start(out=bias_sb[0:1, :], in_=bias[:])
                ones_sb = cpool.tile([128, 128], fp, tag="ones")
                nc.vector.memset(ones_sb[:], 0.0)
                nc.vector.memset(ones_sb[0:1, :], 1.0)

            vs = [None] * NBLK

            def proj(m):
                # v[m] = s[m] * (x[m] @ W.T + b)
                pp = ppsum.tile([128, D], fp, tag="pp")
                nc.tensor.matmul(
                    pp[:], xT_sb[0][:, ts(m, 128)], wT_sb[:, 0:D],
                    start=True, stop=False,
                )
                nc.tensor.matmul(
                    pp[:], xT_sb[1][:, ts(m, 128)], wT_sb[:, D:2 * D],
                    start=False, stop=(BIAS_MODE == "none"),
                )
                if BIAS_MODE != "none":
                    nc.tensor.matmul(
                        pp[:], ones_sb[:], bias_sb[:], start=False, stop=True,
                    )
                v = vpool.tile([128, D], fp, tag="v")
                nc.vector.tensor_scalar_mul(v[:], pp[:], s_sb[:, m:m + 1])
                vs[m] = v

            def decay(m):
                # out rows ts(m-1) = A_L @ v[m-1] + A_0 @ v[m] + A_R @ v[m+1]
                dp = dpsum.tile([128, D], fp, tag="dp")
                nc.tensor.matmul(dp[:], md_sb[:, 0:128], vs[m - 1][:],
                                 start=True, stop=False)
                nc.tensor.matmul(dp[:], md_sb[:, 128:256], vs[m][:],
                                 start=False, stop=False)
                nc.tensor.matmul(dp[:], md_sb[:, 256:384], vs[m + 1][:],
                                 start=False, stop=True)
                ob = opool.tile([128, D], fp, tag="ob")
                nc.vector.tensor_copy(ob[:], dp[:])
                nc.sync.dma_start(out=out[ts(m - 1, 128), :], in_=ob[:])

            proj(0)
            proj(1)
            for m in range(1, NBLK - 1):
                proj(m + 1)
                decay(m)

    nc.compile()
    return nc


def _shard_inputs(x, padding_mask, W_in, b_in):
    x = np.asarray(x, np.float32)
    padding_mask = np.asarray(padding_mask, np.float32)
    wT = np.ascontiguousarray(np.asarray(W_in, np.float32).T).reshape(2, 128, D)
    bias = np.asarray(b_in, np.float32).reshape(1, D)
    in_maps = []
    for c in range(NCORES):
        bidx, half = divmod(c, 2)
        start = half * THALF
        lo, hi = start - HALO, start + THALF + HALO
        glo, ghi = max(lo, 0), min(hi, T)
        xsl = np.zeros((LOC, D), np.float32)
        xsl[glo - lo:ghi - lo] = x[bidx, glo:ghi]
        xTc = np.ascontiguousarray(xsl.T).reshape(2, 128, LOC)
        svec = np.zeros((LOC,), np.float32)
        svec[glo - lo:ghi - lo] = np.exp(
            padding_mask[bidx, 0, glo:ghi] / np.float32(E))
        sc = np.ascontiguousarray(svec.reshape(NBLK, 128).T)
        im = {"xT": xTc, "wT": wT, "bias": bias, "s": sc}
        if not MD_INLINE:
            im["mdecay"] = _decay_blocks()
        in_maps.append(im)
    return in_maps


def kernel(x, padding_mask, W_in, b_in):
    from concourse.bass_utils import run_bass_kernel_spmd

    if "nc" not in _CACHE:
        _CACHE["nc"] = _build()
    nc = _CACHE["nc"]

    in_maps = _shard_inputs(x, padding_mask, W_in, b_in)
    res = run_bass_kernel_spmd(nc, in_maps, list(range(NCORES)))
    out = np.empty((B, T, D), np.float32)
    for c in range(NCORES):
        bidx, half = divmod(c, 2)
        out[bidx, half * THALF:(half + 1) * THALF] = res.results[c]["out"]
    return out


# revision 12
# speedup vs baseline: 1.6903x; 1.6903x over previous
"""DistanceAttention Trainium2 kernel.

Computes, for x:[B,T,D]:
    v    = x @ W_in.T + b_in
    attn = exp((-|i-j| + padding_mask) / e)        # [B,T,T], no softmax
    out  = attn @ v

Key fact: attn factors as exp(-|i-j|/e) * exp(mask_j/e).  The distance
kernel r^|i-j| (r = exp(-1/e) ~= 0.692) underflows fp32 (< 1e-21) for
|i-j| >= 128, so attn is numerically block-tridiagonal with three
CONSTANT 128x128 blocks shared by every row-block, every batch, every
core.  The t x t matmul collapses to 3 small matmuls per 128-row block.

Sharding: batch(4) x seq-half(2) -> 8 cores, each owning 2048 rows plus
a 128-row halo on each side.  exp(mask/e) folds into a per-row scale on
v; phantom halo rows get scale 0.  No cross-core communication.

The b_in contribution is rank-1 through the attention --
attn @ (1 (x) b) = (attn @ exp(mask/e)) (x) b -- and is added exactly on
the host (b_in is zero in this problem; the path exists for generality).
"""

import numpy as np

B, T, D = 4, 4096, 256
NCORES = 8
THALF = T // 2  # rows owned per core
HALO = 128
LOC = THALF + 2 * HALO  # local rows incl. halo
NBLK = LOC // 128  # 18 local 128-row blocks
E = float(np.e)

# "f32r" streams fp32 data through the PE in single-pass mode (4x the
# throughput of the 2-pass fp32 decomposition); "f32" is the safe path.
MM_DTYPE = "f32r"

_CACHE: dict = {}


def _decay_blocks() -> np.ndarray:
    """lhsT-layout decay blocks [128, 3*128]: L | 0 | R.

    matmul(out, lhsT, rhs) computes out[p,n] = sum_q lhsT[q,p] rhs[q,n].
    Out-block m needs  A_L @ v[m-1] + A_0 @ v[m] + A_R @ v[m+1]  with
      A_L[p,q] = r^(128+p-q),  A_0[p,q] = r^|p-q|,  A_R[p,q] = r^(128+q-p)
    so lhsT_L[q,p] = A_L[p,q] etc.  Entries are computed exactly like the
    reference: exp(-dist/e) in fp32.
    """
    i = np.arange(128, dtype=np.float64)
    dL = 128.0 + i[None, :] - i[:, None]  # lhsT_L[a,b] = r^(128+b-a)
    d0 = np.abs(i[:, None] - i[None, :])
    dR = 128.0 + i[:, None] - i[None, :]  # lhsT_R[a,b] = r^(128+a-b)
    dist = np.concatenate([dL, d0, dR], axis=1)
    tg = (-dist.astype(np.float32)) / np.float32(E)
    return np.exp(tg).astype(np.float32)


def _build():
    import concourse.bacc as bacc
    import concourse.mybir as mybir
    from concourse.bass import ts
    from concourse.tile import TileContext

    fp = mybir.dt.float32
    mmdt = mybir.dt.float32r if MM_DTYPE == "f32r" else mybir.dt.float32

    def mm(ap):
        return ap

    nc = bacc.Bacc(None, target_bir_lowering=False, debug=False)

    xT = nc.dram_tensor("xT", [2, 128, LOC], mmdt, kind="ExternalInput")
    wT = nc.dram_tensor("wT", [2, 128, D], mmdt, kind="ExternalInput")
    s = nc.dram_tensor("s", [128, NBLK], fp, kind="ExternalInput")
    out = nc.dram_tensor("out", [THALF, D], fp, kind="ExternalOutput")
    md = nc.dram_tensor("mdecay", [128, 3 * 128], mmdt, kind="ExternalInput")

    with TileContext(nc) as tc:
        with (
            tc.tile_pool(name="const", bufs=1) as cpool,
            tc.tile_pool(name="vpool", bufs=1) as vpool,
            tc.tile_pool(name="opool", bufs=3) as opool,
            tc.tile_pool(name="ppsum", bufs=3, space="PSUM") as ppsum,
            tc.tile_pool(name="dpsum", bufs=4, space="PSUM") as dpsum,
        ):
            xT_sb = []
            for k in range(2):
                t = cpool.tile([128, LOC], mmdt, tag=f"xT{k}")
                nc.sync.dma_start(out=t[:], in_=xT[k])
                xT_sb.append(t)
            wT_sb = cpool.tile([128, 2 * D], mmdt, tag="wT")
            for k in range(2):
                nc.sync.dma_start(out=wT_sb[:, ts(k, D)], in_=wT[k])
            s_sb = cpool.tile([128, NBLK], fp, tag="s")
            nc.sync.dma_start(out=s_sb[:], in_=s[:])
            md_sb = cpool.tile([128, 3 * 128], mmdt, tag="md")
            nc.sync.dma_start(out=md_sb[:], in_=md[:])

            # all 18 v blocks in one tile so decay pairs can read any
            # 512-wide window [v_a | v_a+1] contiguously
            v_sb = vpool.tile([128, NBLK * D], mmdt, tag="v")

            for m in range(NBLK):
                # v[m] = s[m] * (x[m] @ W.T)
                pp = ppsum.tile([128, D], fp, tag="pp")
                nc.tensor.matmul(
                    pp[:], mm(xT_sb[0][:, ts(m, 128)]), mm(wT_sb[:, 0:D]),
                    start=True, stop=False,
                )
                nc.tensor.matmul(
                    pp[:], mm(xT_sb[1][:, ts(m, 128)]), mm(wT_sb[:, D:2 * D]),
                    start=False, stop=True,
                )
                nc.vector.tensor_scalar_mul(
                    v_sb[:, ts(m, D)], pp[:], s_sb[:, m:m + 1])

            for a in range(1, NBLK - 2, 2):
                # out blocks (a, a+1) as one [128, 512] PSUM pair:
                # each diagonal's weights apply to both halves at once
                dp = dpsum.tile([128, 2 * D], fp, tag="dp")
                nc.tensor.matmul(dp[:], mm(md_sb[:, 0:128]),
                                 mm(v_sb[:, (a - 1) * D:(a + 1) * D]),
                                 start=True, stop=False)
                nc.tensor.matmul(dp[:], mm(md_sb[:, 128:256]),
                                 mm(v_sb[:, a * D:(a + 2) * D]),
                                 start=False, stop=False)
                nc.tensor.matmul(dp[:], mm(md_sb[:, 256:384]),
                                 mm(v_sb[:, (a + 1) * D:(a + 3) * D]),
                                 start=False, stop=True)
                ob = opool.tile([128, 2 * D], fp, tag="ob")
                nc.vector.tensor_copy(ob[:], dp[:])
                nc.sync.dma_start(
                    out=out.rearrange("(n p) d -> p n d", p=128)[:, a - 1:a + 1, :],
                    in_=ob[:].rearrange("p (n d) -> p n d", n=2),
                )

    nc.compile()
    return nc


def _shard_inputs(x, padding_mask, W_in, b_in):
    x = np.asarray(x, np.float32)
    padding_mask = np.asarray(padding_mask, np.float32)
    wT = np.ascontiguousarray(np.asarray(W_in, np.float32).T).reshape(2, 128, D)
    in_maps = []
    for c in range(NCORES):
        bidx, half = divmod(c, 2)
        start = half * THALF
        lo, hi = start - HALO, start + THALF + HALO
        glo, ghi = max(lo, 0), min(hi, T)
        xsl = np.zeros((LOC, D), np.float32)
        xsl[glo - lo:ghi - lo] = x[bidx, glo:ghi]
        xTc = np.ascontiguousarray(xsl.T).reshape(2, 128, LOC)
        svec = np.zeros((LOC,), np.float32)
        svec[glo - lo:ghi - lo] = np.exp(
            padding_mask[bidx, 0, glo:ghi] / np.float32(E))
        sc = np.ascontiguousarray(svec.reshape(NBLK, 128).T)
        in_maps.append({"xT": xTc, "wT": wT, "s": sc, "mdecay": _decay_blocks()})
    return in_maps


def _bias_correction(out, padding_mask, b_in):
    """out += attn @ (1 (x) b_in) = (attn_dist @ exp(mask/e)) (x) b_in."""
    b_in = np.asarray(b_in, np.float32)
    if not np.any(b_in):
        return
    k = np.arange(-256, 257, dtype=np.float32)
    w = np.exp(-np.abs(k) / np.float32(E)).astype(np.float64)
    s_all = np.exp(np.asarray(padding_mask, np.float32)[:, 0, :]
                   / np.float32(E)).astype(np.float64)
    for bidx in range(B):
        a = np.convolve(s_all[bidx], w, mode="same").astype(np.float32)
        out[bidx] += np.outer(a, b_in)


def kernel(x, padding_mask, W_in, b_in):
    from concourse.bass_utils import run_bass_kernel_spmd

    if "nc" not in _CACHE:
        _CACHE["nc"] = _build()
    nc = _CACHE["nc"]

    in_maps = _shard_inputs(x, padding_mask, W_in, b_in)
    res = run_bass_kernel_spmd(nc, in_maps, list(range(NCORES)))
    out = np.empty((B, T, D), np.float32)
    for c in range(NCORES):
        bidx, half = divmod(c, 2)
        out[bidx, half * THALF:(half + 1) * THALF] = res.results[c]["out"]
    _bias_correction(out, padding_mask, b_in)
    return out


# revision 16
# speedup vs baseline: 1.8964x; 1.1220x over previous
"""DistanceAttention Trainium2 kernel.

Computes, for x:[B,T,D]:
    v    = x @ W_in.T + b_in
    attn = exp((-|i-j| + padding_mask) / e)        # [B,T,T], no softmax
    out  = attn @ v

Key facts exploited:
  * attn factors as exp(-|i-j|/e) * exp(mask_j/e).  The distance kernel
    r^|i-j| (r = exp(-1/e) ~= 0.692) underflows fp32 (< 1e-21) for
    |i-j| >= 128, so attn is numerically block-tridiagonal with three
    CONSTANT 128x128 blocks shared by every row-block/batch/core: the
    t x t matmul collapses to 3 small matmuls per 128-row block.
  * exp(mask/e) is a per-row scale of v and commutes with the
    projection: it is folded into x on the host.  Phantom halo rows are
    zero-padded, which the same mechanism handles.
  * b_in enters the output as (attn @ exp(mask/e)) (x) b_in -- a rank-1
    term added exactly on the host (b_in is zero here; generality path).

Sharding: batch(4) x seq-half(2) -> 8 cores, each owning 2048 rows plus
a 128-row halo per side.  No cross-core communication.
"""

import numpy as np

B, T, D = 4, 4096, 256
NCORES = 8
THALF = T // 2  # rows owned per core
HALO = 128
LOC = THALF + 2 * HALO  # local rows incl. halo
NBLK = LOC // 128  # 18 local 128-row blocks
NCH = 3  # xT DMA chunks per k-half (finer => earlier PE start)
CHB = NBLK // NCH  # blocks per chunk
E = float(np.e)

# "f32r" streams fp32 data through the PE in single-pass mode (4x the
# throughput of the 2-pass fp32 decomposition); "f32" is the safe path.
MM_DTYPE = "f32r"
OUT_VIA = "copy"  # PSUM is not a legal DMA endpoint; route via SBUF

_CACHE: dict = {}


def _decay_blocks() -> np.ndarray:
    """lhsT-layout decay blocks [128, 3*128]: L | 0 | R.

    matmul(out, lhsT, rhs) computes out[p,n] = sum_q lhsT[q,p] rhs[q,n].
    Out-block m needs  A_L @ v[m-1] + A_0 @ v[m] + A_R @ v[m+1]  with
      A_L[p,q] = r^(128+p-q),  A_0[p,q] = r^|p-q|,  A_R[p,q] = r^(128+q-p)
    so lhsT_L[q,p] = A_L[p,q] etc.  Entries are computed exactly like the
    reference: exp(-dist/e) in fp32.
    """
    i = np.arange(128, dtype=np.float64)
    dL = 128.0 + i[None, :] - i[:, None]  # lhsT_L[a,b] = r^(128+b-a)
    d0 = np.abs(i[:, None] - i[None, :])
    dR = 128.0 + i[:, None] - i[None, :]  # lhsT_R[a,b] = r^(128+a-b)
    dist = np.concatenate([dL, d0, dR], axis=1)
    tg = (-dist.astype(np.float32)) / np.float32(E)
    return np.exp(tg).astype(np.float32)


def _build():
    import concourse.bacc as bacc
    import concourse.mybir as mybir
    from concourse.bass import ts
    from concourse.tile import TileContext

    fp = mybir.dt.float32
    mmdt = mybir.dt.float32r if MM_DTYPE == "f32r" else mybir.dt.float32

    nc = bacc.Bacc(None, target_bir_lowering=False, debug=False)

    # xT[k][j]: d-half k, t-chunk j of the (pre-scaled, transposed) x slice
    xT = nc.dram_tensor("xT", [2, NCH, 128, CHB * 128], mmdt,
                        kind="ExternalInput")
    wT = nc.dram_tensor("wT", [2, 128, D], mmdt, kind="ExternalInput")
    out = nc.dram_tensor("out", [THALF, D], fp, kind="ExternalOutput")
    md = nc.dram_tensor("mdecay", [128, 3 * 128], mmdt, kind="ExternalInput")

    with TileContext(nc) as tc:
        with (
            tc.tile_pool(name="const", bufs=1) as cpool,
            tc.tile_pool(name="vpool", bufs=1) as vpool,
            tc.tile_pool(name="opool", bufs=3) as opool,
            tc.tile_pool(name="ppsum", bufs=4, space="PSUM") as ppsum,
            tc.tile_pool(name="dpsum", bufs=4, space="PSUM") as dpsum,
        ):
            # weights + first x chunks first so the PE can start early;
            # later chunks stream behind the first projections
            wT_sb = cpool.tile([128, 2 * D], mmdt, tag="wT")
            for k in range(2):
                nc.sync.dma_start(out=wT_sb[:, ts(k, D)], in_=wT[k])
            xc = [[None] * NCH for _ in range(2)]
            for k in range(2):
                xc[k][0] = cpool.tile([128, CHB * 128], mmdt,
                                      name=f"x{k}0", tag=f"x{k}0")
                nc.sync.dma_start(out=xc[k][0][:], in_=xT[k, 0])
            md_sb = cpool.tile([128, 3 * 128], mmdt, tag="md")
            nc.sync.dma_start(out=md_sb[:], in_=md[:])
            for j in range(1, NCH):
                for k in range(2):
                    xc[k][j] = cpool.tile([128, CHB * 128], mmdt,
                                          name=f"x{k}{j}", tag=f"x{k}{j}")
                    nc.sync.dma_start(out=xc[k][j][:], in_=xT[k, j])

            # all 18 v blocks in one tile so any 512-wide window
            # [v_a | v_a+1] is a contiguous rhs
            v_sb = vpool.tile([128, NBLK * D], mmdt, tag="v")

            def xap(k, m):  # lhsT for t-block m, d-half k
                return xc[k][m // CHB][:, ts(m % CHB, 128)]

            for a in range(0, NBLK, 2):
                # project blocks (a, a+1) into one [128, 512] PSUM pair
                pp = ppsum.tile([128, 2 * D], fp, tag="pp")
                nc.tensor.matmul(pp[:, 0:D], xap(0, a), wT_sb[:, 0:D],
                                 start=True, stop=False)
                nc.tensor.matmul(pp[:, D:2 * D], xap(0, a + 1), wT_sb[:, 0:D],
                                 start=False, stop=False)
                nc.tensor.matmul(pp[:, 0:D], xap(1, a), wT_sb[:, D:2 * D],
                                 start=False, stop=False)
                nc.tensor.matmul(pp[:, D:2 * D], xap(1, a + 1), wT_sb[:, D:2 * D],
                                 start=False, stop=True)
                nc.vector.tensor_copy(v_sb[:, a * D:(a + 2) * D], pp[:])

            for a in range(1, NBLK - 2, 2):
                # out blocks (a, a+1) as one [128, 512] PSUM pair:
                # each diagonal's weights apply to both halves at once
                dp = dpsum.tile([128, 2 * D], fp, tag="dp")
                nc.tensor.matmul(dp[:], md_sb[:, 0:128],
                                 v_sb[:, (a - 1) * D:(a + 1) * D],
                                 start=True, stop=False)
                nc.tensor.matmul(dp[:], md_sb[:, 128:256],
                                 v_sb[:, a * D:(a + 2) * D],
                                 start=False, stop=False)
                nc.tensor.matmul(dp[:], md_sb[:, 256:384],
                                 v_sb[:, (a + 1) * D:(a + 3) * D],
                                 start=False, stop=True)
                dst = out.rearrange("(n p) d -> p n d", p=128)[:, a - 1:a + 1, :]
                if OUT_VIA == "dma":
                    nc.sync.dma_start(
                        out=dst, in_=dp[:].rearrange("p (n d) -> p n d", n=2))
                else:
                    ob = opool.tile([128, 2 * D], fp, tag="ob")
                    nc.vector.tensor_copy(ob[:], dp[:])
                    nc.sync.dma_start(
                        out=dst, in_=ob[:].rearrange("p (n d) -> p n d", n=2))

    nc.compile()
    return nc


def _shard_inputs(x, padding_mask, W_in, b_in):
    x = np.asarray(x, np.float32)
    padding_mask = np.asarray(padding_mask, np.float32)
    if np.any(padding_mask):
        x = x * np.exp(padding_mask / np.float32(E)).transpose(0, 2, 1)
    wT = np.ascontiguousarray(np.asarray(W_in, np.float32).T).reshape(2, 128, D)
    mdec = _decay_blocks()
    in_maps = []
    for c in range(NCORES):
        bidx, half = divmod(c, 2)
        start = half * THALF
        lo, hi = start - HALO, start + THALF + HALO
        glo, ghi = max(lo, 0), min(hi, T)
        xsl = np.zeros((LOC, D), np.float32)
        xsl[glo - lo:ghi - lo] = x[bidx, glo:ghi]
        xTc = np.ascontiguousarray(
            xsl.T.reshape(2, 128, NCH, CHB * 128).transpose(0, 2, 1, 3))
        in_maps.append({"xT": xTc, "wT": wT, "mdecay": mdec})
    return in_maps


def _bias_correction(out, padding_mask, b_in):
    """out += attn @ (1 (x) b_in) = (attn_dist @ exp(mask/e)) (x) b_in."""
    b_in = np.asarray(b_in, np.float32)
    if not np.any(b_in):
        return
    k = np.arange(-256, 257, dtype=np.float32)
    w = np.exp(-np.abs(k) / np.float32(E)).astype(np.float64)
    s_all = np.exp(np.asarray(padding_mask, np.float32)[:, 0, :]
                   / np.float32(E)).astype(np.float64)
    for bidx in range(B):
        a = np.convolve(s_all[bidx], w, mode="same").astype(np.float32)
        out[bidx] += np.outer(a, b_in)


def kernel(x, padding_mask, W_in, b_in):
    from concourse.bass_utils import run_bass_kernel_spmd

    if "nc" not in _CACHE:
        _CACHE["nc"] = _build()
    nc = _CACHE["nc"]

    in_maps = _shard_inputs(x, padding_mask, W_in, b_in)
    res = run_bass_kernel_spmd(nc, in_maps, list(range(NCORES)))
    out = np.empty((B, T, D), np.float32)
    for c in range(NCORES):
        bidx, half = divmod(c, 2)
        out[bidx, half * THALF:(half + 1) * THALF] = res.results[c]["out"]
    _bias_correction(out, padding_mask, b_in)
    return out


# revision 18
# speedup vs baseline: 1.9412x; 1.0236x over previous
"""DistanceAttention Trainium2 kernel.

Computes, for x:[B,T,D]:
    v    = x @ W_in.T + b_in
    attn = exp((-|i-j| + padding_mask) / e)        # [B,T,T], no softmax
    out  = attn @ v

Key facts exploited:
  * attn factors as exp(-|i-j|/e) * exp(mask_j/e).  The distance kernel
    r^|i-j| (r = exp(-1/e) ~= 0.692) underflows fp32 (< 1e-21) for
    |i-j| >= 128, so attn is numerically block-tridiagonal with three
    CONSTANT 128x128 blocks shared by every row-block/batch/core: the
    t x t matmul collapses to 3 small matmuls per 128-row block.
  * exp(mask/e) is a per-row scale of v and commutes with the
    projection: it is folded into x on the host.  Phantom halo rows are
    zero-padded, which the same mechanism handles.
  * b_in enters the output as (attn @ exp(mask/e)) (x) b_in -- a rank-1
    term added exactly on the host (b_in is zero here; generality path).

Sharding: batch(4) x seq-half(2) -> 8 cores, each owning 2048 rows plus
a 128-row halo per side.  No cross-core communication.
"""

import numpy as np

B, T, D = 4, 4096, 256
NCORES = 8
THALF = T // 2  # rows owned per core
HALO = 128
LOC = THALF + 2 * HALO  # local rows incl. halo
NBLK = LOC // 128  # 18 local 128-row blocks
NCH = 3  # xT DMA chunks per k-half (finer => earlier PE start)
CHB = NBLK // NCH  # blocks per chunk
E = float(np.e)

# "f32r" streams fp32 data through the PE in single-pass mode (4x the
# throughput of the 2-pass fp32 decomposition); "f32" is the safe path.
MM_DTYPE = "f32r"
OUT_VIA = "copy"  # PSUM is not a legal DMA endpoint; route via SBUF

_CACHE: dict = {}


def _decay_blocks() -> np.ndarray:
    """lhsT-layout decay blocks [128, 3*128]: L | 0 | R.

    matmul(out, lhsT, rhs) computes out[p,n] = sum_q lhsT[q,p] rhs[q,n].
    Out-block m needs  A_L @ v[m-1] + A_0 @ v[m] + A_R @ v[m+1]  with
      A_L[p,q] = r^(128+p-q),  A_0[p,q] = r^|p-q|,  A_R[p,q] = r^(128+q-p)
    so lhsT_L[q,p] = A_L[p,q] etc.  Entries are computed exactly like the
    reference: exp(-dist/e) in fp32.
    """
    i = np.arange(128, dtype=np.float64)
    dL = 128.0 + i[None, :] - i[:, None]  # lhsT_L[a,b] = r^(128+b-a)
    d0 = np.abs(i[:, None] - i[None, :])
    dR = 128.0 + i[:, None] - i[None, :]  # lhsT_R[a,b] = r^(128+a-b)
    dist = np.concatenate([dL, d0, dR], axis=1)
    tg = (-dist.astype(np.float32)) / np.float32(E)
    return np.exp(tg).astype(np.float32)


def _build():
    import concourse.bacc as bacc
    import concourse.mybir as mybir
    from concourse.bass import ts
    from concourse.tile import TileContext

    fp = mybir.dt.float32
    mmdt = mybir.dt.float32r if MM_DTYPE == "f32r" else mybir.dt.float32

    nc = bacc.Bacc(None, target_bir_lowering=False, debug=False)

    # xT[k][j]: d-half k, t-chunk j of the (pre-scaled, transposed) x slice
    xT = nc.dram_tensor("xT", [2, NCH, 128, CHB * 128], mmdt,
                        kind="ExternalInput")
    wT = nc.dram_tensor("wT", [2, 128, D], mmdt, kind="ExternalInput")
    out = nc.dram_tensor("out", [THALF, D], fp, kind="ExternalOutput")
    md = nc.dram_tensor("mdecay", [128, 3 * 128], mmdt, kind="ExternalInput")

    with TileContext(nc) as tc:
        with (
            tc.tile_pool(name="const", bufs=1) as cpool,
            tc.tile_pool(name="vpool", bufs=1) as vpool,
            tc.tile_pool(name="opool", bufs=3) as opool,
            tc.tile_pool(name="ppsum", bufs=4, space="PSUM") as ppsum,
            tc.tile_pool(name="dpsum", bufs=4, space="PSUM") as dpsum,
        ):
            # DMA order = dependency order of the first matmuls: the
            # k=0 weight half + first x chunk unblock the PE; everything
            # else streams behind the first projections
            wT_sb = [None, None]
            xc = [[None] * NCH for _ in range(2)]
            for k in range(2):
                wT_sb[k] = cpool.tile([128, D], mmdt,
                                      name=f"w{k}", tag=f"w{k}")
                nc.sync.dma_start(out=wT_sb[k][:], in_=wT[k])
                xc[k][0] = cpool.tile([128, CHB * 128], mmdt,
                                      name=f"x{k}0", tag=f"x{k}0")
                nc.sync.dma_start(out=xc[k][0][:], in_=xT[k, 0])
            md_sb = cpool.tile([128, 3 * 128], mmdt, tag="md")
            nc.gpsimd.dma_start(out=md_sb[:], in_=md[:])
            for j in range(1, NCH):
                for k in range(2):
                    xc[k][j] = cpool.tile([128, CHB * 128], mmdt,
                                          name=f"x{k}{j}", tag=f"x{k}{j}")
                    nc.sync.dma_start(out=xc[k][j][:], in_=xT[k, j])

            # all 18 v blocks in one tile so any 512-wide window
            # [v_a | v_a+1] is a contiguous rhs
            v_sb = vpool.tile([128, NBLK * D], mmdt, tag="v")

            def xap(k, m):  # lhsT for t-block m, d-half k
                return xc[k][m // CHB][:, ts(m % CHB, 128)]

            def proj_pair(p):
                # project blocks (2p, 2p+1) into one [128, 512] PSUM pair
                a = 2 * p
                pp = ppsum.tile([128, 2 * D], fp, tag="pp")
                nc.tensor.matmul(pp[:, 0:D], xap(0, a), wT_sb[0][:],
                                 start=True, stop=False)
                nc.tensor.matmul(pp[:, D:2 * D], xap(0, a + 1), wT_sb[0][:],
                                 start=False, stop=False)
                nc.tensor.matmul(pp[:, 0:D], xap(1, a), wT_sb[1][:],
                                 start=False, stop=False)
                nc.tensor.matmul(pp[:, D:2 * D], xap(1, a + 1), wT_sb[1][:],
                                 start=False, stop=True)
                nc.vector.tensor_copy(v_sb[:, a * D:(a + 2) * D], pp[:])

            def decay_pair(a):
                # out blocks (a, a+1) as one [128, 512] PSUM pair:
                # each diagonal's weights apply to both halves at once
                dp = dpsum.tile([128, 2 * D], fp, tag="dp")
                nc.tensor.matmul(dp[:], md_sb[:, 0:128],
                                 v_sb[:, (a - 1) * D:(a + 1) * D],
                                 start=True, stop=False)
                nc.tensor.matmul(dp[:], md_sb[:, 128:256],
                                 v_sb[:, a * D:(a + 2) * D],
                                 start=False, stop=False)
                nc.tensor.matmul(dp[:], md_sb[:, 256:384],
                                 v_sb[:, (a + 1) * D:(a + 3) * D],
                                 start=False, stop=True)
                dst = out.rearrange("(n p) d -> p n d", p=128)[:, a - 1:a + 1, :]
                ob = opool.tile([128, 2 * D], fp, tag="ob")
                nc.vector.tensor_copy(ob[:], dp[:])
                nc.sync.dma_start(
                    out=dst, in_=ob[:].rearrange("p (n d) -> p n d", n=2))

            # interleave: decay pair a=2k+1 (v blocks a-1..a+2) becomes
            # ready right after proj pair k+1 -- emit it there so its
            # copy/DMA drain while later projections still run
            proj_pair(0)
            proj_pair(1)
            decay_pair(1)
            for p in range(2, NBLK // 2):
                proj_pair(p)
                decay_pair(2 * p - 1)

    nc.compile()
    return nc


def _shard_inputs(x, padding_mask, W_in, b_in):
    x = np.asarray(x, np.float32)
    padding_mask = np.asarray(padding_mask, np.float32)
    if np.any(padding_mask):
        x = x * np.exp(padding_mask / np.float32(E)).transpose(0, 2, 1)
    wT = np.ascontiguousarray(np.asarray(W_in, np.float32).T).reshape(2, 128, D)
    mdec = _decay_blocks()
    in_maps = []
    for c in range(NCORES):
        bidx, half = divmod(c, 2)
        start = half * THALF
        lo, hi = start - HALO, start + THALF + HALO
        glo, ghi = max(lo, 0), min(hi, T)
        xsl = np.zeros((LOC, D), np.float32)
        xsl[glo - lo:ghi - lo] = x[bidx, glo:ghi]
        xTc = np.ascontiguousarray(
            xsl.T.reshape(2, 128, NCH, CHB * 128).transpose(0, 2, 1, 3))
        in_maps.append({"xT": xTc, "wT": wT, "mdecay": mdec})
    return in_maps


def _bias_correction(out, padding_mask, b_in):
    """out += attn @ (1 (x) b_in) = (attn_dist @ exp(mask/e)) (x) b_in."""
    b_in = np.asarray(b_in, np.float32)
    if not np.any(b_in):
        return
    k = np.arange(-256, 257, dtype=np.float32)
    w = np.exp(-np.abs(k) / np.float32(E)).astype(np.float64)
    s_all = np.exp(np.asarray(padding_mask, np.float32)[:, 0, :]
                   / np.float32(E)).astype(np.float64)
    for bidx in range(B):
        a = np.convolve(s_all[bidx], w, mode="same").astype(np.float32)
        out[bidx] += np.outer(a, b_in)


def kernel(x, padding_mask, W_in, b_in):
    from concourse.bass_utils import run_bass_kernel_spmd

    if "nc" not in _CACHE:
        _CACHE["nc"] = _build()
    nc = _CACHE["nc"]

    in_maps = _shard_inputs(x, padding_mask, W_in, b_in)
    res = run_bass_kernel_spmd(nc, in_maps, list(range(NCORES)))
    out = np.empty((B, T, D), np.float32)
    for c in range(NCORES):
        bidx, half = divmod(c, 2)
        out[bidx, half * THALF:(half + 1) * THALF] = res.results[c]["out"]
    _bias_correction(out, padding_mask, b_in)
    return out


# revision 19
# speedup vs baseline: 2.0077x; 1.0342x over previous
"""DistanceAttention Trainium2 kernel.

Computes, for x:[B,T,D]:
    v    = x @ W_in.T + b_in
    attn = exp((-|i-j| + padding_mask) / e)        # [B,T,T], no softmax
    out  = attn @ v

Key facts exploited:
  * attn factors as exp(-|i-j|/e) * exp(mask_j/e).  The distance kernel
    r^|i-j| (r = exp(-1/e) ~= 0.692) underflows fp32 (< 1e-21) for
    |i-j| >= 128, so attn is numerically block-tridiagonal with three
    CONSTANT 128x128 blocks shared by every row-block/batch/core: the
    t x t matmul collapses to 3 small matmuls per 128-row block.
  * exp(mask/e) is a per-row scale of v and commutes with the
    projection: it is folded into x on the host.  Phantom halo rows are
    zero-padded, which the same mechanism handles.
  * b_in enters the output as (attn @ exp(mask/e)) (x) b_in -- a rank-1
    term added exactly on the host (b_in is zero here; generality path).

Sharding: batch(4) x seq-half(2) -> 8 cores, each owning 2048 rows plus
a 128-row halo per side.  No cross-core communication.
"""

import numpy as np

B, T, D = 4, 4096, 256
NCORES = 8
THALF = T // 2  # rows owned per core
HALO = 128
LOC = THALF + 2 * HALO  # local rows incl. halo
NBLK = LOC // 128  # 18 local 128-row blocks
NCH = 3  # xT DMA chunks per k-half (finer => earlier PE start)
CHB = NBLK // NCH  # blocks per chunk
E = float(np.e)

# "f32r" streams fp32 data through the PE in single-pass mode (4x the
# throughput of the 2-pass fp32 decomposition); "f32" is the safe path.
MM_DTYPE = "f32r"
OUT_VIA = "copy"  # PSUM is not a legal DMA endpoint; route via SBUF

_CACHE: dict = {}


def _decay_blocks() -> np.ndarray:
    """lhsT-layout decay blocks [128, 3*128]: L | 0 | R.

    matmul(out, lhsT, rhs) computes out[p,n] = sum_q lhsT[q,p] rhs[q,n].
    Out-block m needs  A_L @ v[m-1] + A_0 @ v[m] + A_R @ v[m+1]  with
      A_L[p,q] = r^(128+p-q),  A_0[p,q] = r^|p-q|,  A_R[p,q] = r^(128+q-p)
    so lhsT_L[q,p] = A_L[p,q] etc.  Entries are computed exactly like the
    reference: exp(-dist/e) in fp32.
    """
    i = np.arange(128, dtype=np.float64)
    dL = 128.0 + i[None, :] - i[:, None]  # lhsT_L[a,b] = r^(128+b-a)
    d0 = np.abs(i[:, None] - i[None, :])
    dR = 128.0 + i[:, None] - i[None, :]  # lhsT_R[a,b] = r^(128+a-b)
    dist = np.concatenate([dL, d0, dR], axis=1)
    tg = (-dist.astype(np.float32)) / np.float32(E)
    return np.exp(tg).astype(np.float32)


def _build():
    import concourse.bacc as bacc
    import concourse.mybir as mybir
    from concourse.bass import ts
    from concourse.tile import TileContext

    fp = mybir.dt.float32
    mmdt = mybir.dt.float32r if MM_DTYPE == "f32r" else mybir.dt.float32

    nc = bacc.Bacc(None, target_bir_lowering=False, debug=False)

    # xT[k][j]: d-half k, t-chunk j of the (pre-scaled, transposed) x slice
    xT = nc.dram_tensor("xT", [2, NCH, 128, CHB * 128], mmdt,
                        kind="ExternalInput")
    wT = nc.dram_tensor("wT", [2, 128, D], mmdt, kind="ExternalInput")
    out = nc.dram_tensor("out", [THALF, D], fp, kind="ExternalOutput")
    md = nc.dram_tensor("mdecay", [128, 3 * 128], mmdt, kind="ExternalInput")

    with TileContext(nc) as tc:
        with (
            tc.tile_pool(name="const", bufs=1) as cpool,
            tc.tile_pool(name="vpool", bufs=1) as vpool,
            tc.tile_pool(name="opool", bufs=3) as opool,
            tc.tile_pool(name="ppsum", bufs=5, space="PSUM") as ppsum,
            tc.tile_pool(name="dpsum", bufs=3, space="PSUM") as dpsum,
        ):
            # DMA order = dependency order of the first matmuls: the
            # k=0 weight half + first x chunk unblock the PE; everything
            # else streams behind the first projections
            wT_sb = [None, None]
            xc = [[None] * NCH for _ in range(2)]
            for k in range(2):
                wT_sb[k] = cpool.tile([128, D], mmdt,
                                      name=f"w{k}", tag=f"w{k}")
                nc.sync.dma_start(out=wT_sb[k][:], in_=wT[k])
                xc[k][0] = cpool.tile([128, CHB * 128], mmdt,
                                      name=f"x{k}0", tag=f"x{k}0")
                nc.sync.dma_start(out=xc[k][0][:], in_=xT[k, 0])
            md_sb = cpool.tile([128, 3 * 128], mmdt, tag="md")
            nc.sync.dma_start(out=md_sb[:], in_=md[:])
            for j in range(1, NCH):
                for k in range(2):
                    xc[k][j] = cpool.tile([128, CHB * 128], mmdt,
                                          name=f"x{k}{j}", tag=f"x{k}{j}")
                    nc.sync.dma_start(out=xc[k][j][:], in_=xT[k, j])

            # all 18 v blocks in one tile so any 512-wide window
            # [v_a | v_a+1] is a contiguous rhs
            v_sb = vpool.tile([128, NBLK * D], mmdt, tag="v")

            def xap(k, m):  # lhsT for t-block m, d-half k
                return xc[k][m // CHB][:, ts(m % CHB, 128)]

            def proj_pair(p):
                # project blocks (2p, 2p+1) into one [128, 512] PSUM pair
                a = 2 * p
                pp = ppsum.tile([128, 2 * D], fp, tag="pp")
                nc.tensor.matmul(pp[:, 0:D], xap(0, a), wT_sb[0][:],
                                 start=True, stop=False)
                nc.tensor.matmul(pp[:, D:2 * D], xap(0, a + 1), wT_sb[0][:],
                                 start=False, stop=False)
                nc.tensor.matmul(pp[:, 0:D], xap(1, a), wT_sb[1][:],
                                 start=False, stop=False)
                nc.tensor.matmul(pp[:, D:2 * D], xap(1, a + 1), wT_sb[1][:],
                                 start=False, stop=True)
                nc.vector.tensor_copy(v_sb[:, a * D:(a + 2) * D], pp[:])

            def decay_pair(a):
                # out blocks (a, a+1) as one [128, 512] PSUM pair:
                # each diagonal's weights apply to both halves at once
                dp = dpsum.tile([128, 2 * D], fp, tag="dp")
                nc.tensor.matmul(dp[:], md_sb[:, 0:128],
                                 v_sb[:, (a - 1) * D:(a + 1) * D],
                                 start=True, stop=False)
                nc.tensor.matmul(dp[:], md_sb[:, 128:256],
                                 v_sb[:, a * D:(a + 2) * D],
                                 start=False, stop=False)
                nc.tensor.matmul(dp[:], md_sb[:, 256:384],
                                 v_sb[:, (a + 1) * D:(a + 3) * D],
                                 start=False, stop=True)
                dst = out.rearrange("(n p) d -> p n d", p=128)[:, a - 1:a + 1, :]
                ob = opool.tile([128, 2 * D], fp, tag="ob")
                nc.vector.tensor_copy(ob[:], dp[:])
                nc.sync.dma_start(
                    out=dst, in_=ob[:].rearrange("p (n d) -> p n d", n=2))

            # interleave: decay pair a=2k+1 (v blocks a-1..a+2) becomes
            # ready right after proj pair k+1 -- emit it there so its
            # copy/DMA drain while later projections still run
            proj_pair(0)
            proj_pair(1)
            decay_pair(1)
            for p in range(2, NBLK // 2):
                proj_pair(p)
                decay_pair(2 * p - 1)

    nc.compile()
    return nc


def _shard_inputs(x, padding_mask, W_in, b_in):
    x = np.asarray(x, np.float32)
    padding_mask = np.asarray(padding_mask, np.float32)
    if np.any(padding_mask):
        x = x * np.exp(padding_mask / np.float32(E)).transpose(0, 2, 1)
    wT = np.ascontiguousarray(np.asarray(W_in, np.float32).T).reshape(2, 128, D)
    mdec = _decay_blocks()
    in_maps = []
    for c in range(NCORES):
        bidx, half = divmod(c, 2)
        start = half * THALF
        lo, hi = start - HALO, start + THALF + HALO
        glo, ghi = max(lo, 0), min(hi, T)
        xsl = np.zeros((LOC, D), np.float32)
        xsl[glo - lo:ghi - lo] = x[bidx, glo:ghi]
        xTc = np.ascontiguousarray(
            xsl.T.reshape(2, 128, NCH, CHB * 128).transpose(0, 2, 1, 3))
        in_maps.append({"xT": xTc, "wT": wT, "mdecay": mdec})
    return in_maps


def _bias_correction(out, padding_mask, b_in):
    """out += attn @ (1 (x) b_in) = (attn_dist @ exp(mask/e)) (x) b_in."""
    b_in = np.asarray(b_in, np.float32)
    if not np.any(b_in):
        return
    k = np.arange(-256, 257, dtype=np.float32)
    w = np.exp(-np.abs(k) / np.float32(E)).astype(np.float64)
    s_all = np.exp(np.asarray(padding_mask, np.float32)[:, 0, :]
                   / np.float32(E)).astype(np.float64)
    for bidx in range(B):
        a = np.convolve(s_all[bidx], w, mode="same").astype(np.float32)
        out[bidx] += np.outer(a, b_in)


def kernel(x, padding_mask, W_in, b_in):
    from concourse.bass_utils import run_bass_kernel_spmd

    if "nc" not in _CACHE:
        _CACHE["nc"] = _build()
    nc = _CACHE["nc"]

    in_maps = _shard_inputs(x, padding_mask, W_in, b_in)
    res = run_bass_kernel_spmd(nc, in_maps, list(range(NCORES)))
    out = np.empty((B, T, D), np.float32)
    for c in range(NCORES):
        bidx, half = divmod(c, 2)
        out[bidx, half * THALF:(half + 1) * THALF] = res.results[c]["out"]
    _bias_correction(out, padding_mask, b_in)
    return out
